# revision 3
# baseline (speedup 1.0000x reference)
"""Trainium2 Bass kernel for nn_PointCloudGenerator (neural-SDF sphere tracing).

Strategy
--------
Data-parallel over rays: 2 views x 128x128 rays = 32768 rays, sharded 4096
per NeuronCore across 8 cores.  The MLP is tiny and replicated.

Algorithmic folding done on the host (inside kernel(), plain numpy):
  * The 484-dim latent is constant across rays, so lat @ W1[3:,:] folds into
    the layer-1 bias:  b1' = b1 + lat @ W1[3:].  Layer 1 collapses from a
    487x256 matmul to a 4x256 one (3 pos dims + a ones-row carrying b1').
  * Only W3[:,0] (the SDF head) is ever used - the color head is dead code.
  * Everything sdf-related is pre-scaled by SCALE=sqrt(2) so the advance
    step needs no extra multiply:  ssdf = SCALE*sdf, threshold SCALE*eps.

On-device layout is feature-major: activations live as [features, rays]
tiles so W1/W2/W3 feed the PE array directly as stationary operands with
zero transposes anywhere.
"""

import os
import sys

import numpy as np

# ---------------------------------------------------------------- constants
PU = PV = 128
N_RAYS = PU * PV
FOCAL = 1.0
CAM_DIST = 2.2
MAX_RAY = 3.5
EPS = 1e-3
SCALE = float(np.sqrt(2.0))
STEPS = 6
HID = 256
SPHERE_R = 0.7

N_CORES = 8
R_TOTAL = 2 * N_RAYS          # 32768 rays over both camera angles
R_CORE = R_TOTAL // N_CORES   # 4096
R_TILE = 512
N_TILES = R_CORE // R_TILE    # 8

SEPS = SCALE * EPS

for _p in ("/opt/trn_rl_repo", "/root/.axon_site/_ro/trn_rl_repo"):
    if os.path.isdir(_p) and _p not in sys.path:
        sys.path.append(_p)

_PROG_CACHE = {}


# ---------------------------------------------------------------- host math
def _rays_numpy(rho_deg):
    """Mirror of reference._rays in float32 numpy."""
    u = ((np.arange(PU, dtype=np.float32) + 0.5) / np.float32(PU) - np.float32(0.5)) * np.float32(PU / PV)
    v = np.float32(0.5) - (np.arange(PV, dtype=np.float32) + 0.5) / np.float32(PV)
    uu, vv = np.meshgrid(u, v, indexing="ij")
    d = np.stack([uu, vv, np.full_like(uu, -np.float32(FOCAL))], axis=-1).reshape(-1, 3)
    d = d / np.linalg.norm(d, axis=-1, keepdims=True).astype(np.float32)
    th = np.deg2rad(rho_deg)
    c, s = np.float32(np.cos(th)), np.float32(np.sin(th))
    R = np.array([[c, 0.0, s], [0.0, 1.0, 0.0], [-s, 0.0, c]], dtype=np.float32)
    dirs = (d @ R.T).astype(np.float32)
    cam = (R @ np.array([0.0, 0.0, 1.0], np.float32)) * np.float32(CAM_DIST)
    first_step = np.float32(FOCAL + CAM_DIST - 1.0)
    start = (cam[None, :] + first_step * dirs).astype(np.float32)
    return start, dirs


def _host_prep(lat_geo, lat_exp, lat_app, W1, b1, W2, b2, W3, b3):
    """Fold latents/biases; build per-core device input dicts."""
    f = np.float32
    lat = np.concatenate([lat_geo, lat_exp, lat_app]).astype(f)
    b1p = (b1.astype(f) + lat @ W1[3:, :].astype(f)).astype(f)
    # [4, 256]: rows x,y,z of W1 plus the folded bias row (pos row 3 == 1.0)
    w1a4 = np.concatenate([W1[:3, :].astype(f), b1p[None, :]], axis=0).astype(f)
    w3s = (SCALE * W3[:, 0:1].astype(f)).astype(f)          # [256,1]
    cconst = f(SCALE * SPHERE_R - SCALE * float(b3[0]))     # ssdf = s*|pos| - cconst + h2@w3s
    onesr2 = np.array([[SCALE * SCALE]] * 3 + [[0.0]], f)   # [4,1]: r2 matmul weights

    starts, dirs = [], []
    for rho in (-30.0, 30.0):
        st, di = _rays_numpy(rho)
        starts.append(st)
        dirs.append(di)
    start = np.concatenate(starts, 0)   # [32768, 3]
    dirv = np.concatenate(dirs, 0)

    # feature-major with ones / zeros 4th row
    startT = np.concatenate([start.T, np.ones((1, R_TOTAL), f)], 0).astype(f)  # [4, 32768]
    dirT = np.concatenate([dirv.T, np.zeros((1, R_TOTAL), f)], 0).astype(f)

    in_maps = []
    for c in range(N_CORES):
        sl = slice(c * R_CORE, (c + 1) * R_CORE)
        in_maps.append({
            "startT": np.ascontiguousarray(startT[:, sl]),
            "dirT": np.ascontiguousarray(dirT[:, sl]),
            "w1a4": w1a4,
            "w2": np.ascontiguousarray(W2.astype(f)),
            "w3s": w3s,
            "b2d": np.ascontiguousarray(b2.astype(f).reshape(HID, 1)),
            "onesr2": onesr2,
            "cconst": np.array([[cconst]], f),
        })
    return in_maps


# ---------------------------------------------------------------- device build
def _build_program(dt_mode="f32"):
    """Build the per-core Bass program (SPMD: identical on all 8 cores)."""
    from contextlib import ExitStack

    import concourse.bass as bass  # noqa: F401
    import concourse.tile as tile
    from concourse import bacc, mybir
    from concourse import library_config

    f32 = mybir.dt.float32
    AF = mybir.ActivationFunctionType
    OP = mybir.AluOpType

    def mm_ap(ap):
        if dt_mode == "f32r":
            return ap.bitcast(mybir.dt.float32r)
        return ap

    nc = bacc.Bacc("TRN2", target_bir_lowering=False, debug=False,
                   num_devices=N_CORES)

    startT = nc.dram_tensor("startT", [4, R_CORE], f32, kind="ExternalInput").ap()
    dirTd = nc.dram_tensor("dirT", [4, R_CORE], f32, kind="ExternalInput").ap()
    w1a4 = nc.dram_tensor("w1a4", [4, HID], f32, kind="ExternalInput").ap()
    w2 = nc.dram_tensor("w2", [HID, HID], f32, kind="ExternalInput").ap()
    w3s = nc.dram_tensor("w3s", [HID, 1], f32, kind="ExternalInput").ap()
    b2d = nc.dram_tensor("b2d", [HID, 1], f32, kind="ExternalInput").ap()
    onesr2 = nc.dram_tensor("onesr2", [4, 1], f32, kind="ExternalInput").ap()
    cconst = nc.dram_tensor("cconst", [1, 1], f32, kind="ExternalInput").ap()
    outT = nc.dram_tensor("outT", [3, R_CORE], f32, kind="ExternalOutput").ap()

    with tile.TileContext(nc) as tc:
        with ExitStack() as ctx:
            nc.gpsimd.load_library(library_config.attn)

            cpool = ctx.enter_context(tc.tile_pool(name="consts", bufs=1))
            w1a4_s = cpool.tile([4, HID], f32, name="w1a4", tag="w1a4")
            nc.sync.dma_start(w1a4_s[:], w1a4[:])
            w2_s = [[cpool.tile([128, 128], f32, name=f"w2_{i}{j}", tag=f"w2_{i}{j}") for j in range(2)]
                    for i in range(2)]
            for i in range(2):
                for j in range(2):
                    nc.sync.dma_start(w2_s[i][j][:],
                                      w2[128 * i:128 * (i + 1), 128 * j:128 * (j + 1)])
            w3s_s = [cpool.tile([128, 1], f32, name=f"w3s{i}", tag=f"w3s{i}") for i in range(2)]
            b2_s = [cpool.tile([128, 1], f32, name=f"b2{i}", tag=f"b2{i}") for i in range(2)]
            for i in range(2):
                nc.sync.dma_start(w3s_s[i][:], w3s[128 * i:128 * (i + 1), :])
                nc.sync.dma_start(b2_s[i][:], b2d[128 * i:128 * (i + 1), :])
            onesr2_s = cpool.tile([4, 1], f32, name="onesr2", tag="onesr2")
            nc.sync.dma_start(onesr2_s[:], onesr2[:])
            cconst_s = cpool.tile([1, 1], f32, name="cconst", tag="cconst")
            nc.sync.dma_start(cconst_s[:], cconst[:])

            state = ctx.enter_context(tc.tile_pool(name="state", bufs=2))
            dpool = ctx.enter_context(tc.tile_pool(name="dirs", bufs=1))
            pos, tval, act, dirs = {}, {}, {}, {}
            for j in range(N_TILES):
                sl = slice(R_TILE * j, R_TILE * (j + 1))
                p = state.tile([4, R_TILE], f32, name=f"pos{j}", tag=f"pos{j}")
                nc.sync.dma_start(p[:], startT[:, sl])
                d = dpool.tile([4, R_TILE], f32, name=f"dir{j}", tag=f"dir{j}")
                nc.sync.dma_start(d[:], dirTd[:, sl])
                t0 = state.tile([1, R_TILE], f32, name=f"t{j}", tag=f"t{j}")
                nc.vector.memset(t0[:], 0.0)
                a0 = state.tile([1, R_TILE], f32, name=f"act{j}", tag=f"act{j}")
                nc.vector.memset(a0[:], 1.0)
                pos[j], tval[j], act[j], dirs[j] = p, t0, a0, d

            work = ctx.enter_context(tc.tile_pool(name="work", bufs=3))
            psA = ctx.enter_context(tc.tile_pool(name="psA", bufs=2, space="PSUM"))
            psB = ctx.enter_context(tc.tile_pool(name="psB", bufs=2, space="PSUM"))
            psS = ctx.enter_context(tc.tile_pool(name="psS", bufs=2, space="PSUM"))

            for s in range(STEPS + 1):
                last = s == STEPS
                for j in range(N_TILES):
                    p = pos[j]
                    # ---- layer 1: h1 = relu(W1a'.pos + b1')   [256, R]
                    h1 = [work.tile([128, R_TILE], f32, name=f"h1{m}", tag=f"h1{m}") for m in range(2)]
                    for m in range(2):
                        ph1 = psA.tile([128, R_TILE], f32, name="ph1", tag="ph1")
                        nc.tensor.matmul(ph1[:], lhsT=mm_ap(w1a4_s[:, 128 * m:128 * (m + 1)]),
                                         rhs=mm_ap(p[:]), start=True, stop=True)
                        nc.scalar.activation(h1[m][:], ph1[:], AF.Relu)
                    # ---- scaled norm: s*|pos| via sq -> ones-matmul -> sqrt
                    sq = work.tile([4, R_TILE], f32, name="sq", tag="sq")
                    nc.scalar.activation(sq[:], p[:], AF.Square)
                    pr2 = psS.tile([1, R_TILE], f32, name="pr2", tag="pr2")
                    nc.tensor.matmul(pr2[:], lhsT=mm_ap(onesr2_s[:]), rhs=mm_ap(sq[:]),
                                     start=True, stop=True)
                    snorm = work.tile([1, R_TILE], f32, name="snorm", tag="snorm")
                    nc.scalar.activation(snorm[:], pr2[:], AF.Sqrt)
                    # ---- layer 2: h2 = relu(W2.h1 + b2)   [256, R]
                    h2 = [work.tile([128, R_TILE], f32, name=f"h2{m}", tag=f"h2{m}") for m in range(2)]
                    for m in range(2):
                        ph2 = psB.tile([128, R_TILE], f32, name="ph2", tag="ph2")
                        nc.tensor.matmul(ph2[:], lhsT=mm_ap(w2_s[0][m][:]), rhs=mm_ap(h1[0][:]),
                                         start=True, stop=False)
                        nc.tensor.matmul(ph2[:], lhsT=mm_ap(w2_s[1][m][:]), rhs=mm_ap(h1[1][:]),
                                         start=False, stop=True)
                        nc.scalar.activation(h2[m][:], ph2[:], AF.Relu, bias=b2_s[m][:])
                    # ---- layer 3 (sdf head only): po = s*(h2 @ W3[:,0])
                    po = psS.tile([1, R_TILE], f32, name="po", tag="po")
                    nc.tensor.matmul(po[:], lhsT=mm_ap(w3s_s[0][:]), rhs=mm_ap(h2[0][:]),
                                     start=True, stop=False)
                    nc.tensor.matmul(po[:], lhsT=mm_ap(w3s_s[1][:]), rhs=mm_ap(h2[1][:]),
                                     start=False, stop=True)
                    # ---- ssdf = (s*|pos| - cconst) + po
                    ssdf = work.tile([1, R_TILE], f32, name="ssdf", tag="ssdf")
                    nc.vector.scalar_tensor_tensor(ssdf[:], in0=snorm[:], scalar=cconst_s[:],
                                                   in1=po[:], op0=OP.subtract, op1=OP.add)
                    absd = work.tile([1, R_TILE], f32, name="absd", tag="absd")
                    nc.scalar.activation(absd[:], ssdf[:], AF.Abs)
                    if not last:
                        # m = active & ~hit ; adv = m * ssdf
                        mt = work.tile([1, R_TILE], f32, name="mt", tag="mt")
                        nc.vector.scalar_tensor_tensor(mt[:], in0=absd[:], scalar=SEPS,
                                                       in1=act[j][:], op0=OP.is_ge, op1=OP.mult)
                        adv = work.tile([1, R_TILE], f32, name="adv", tag="adv")
                        nc.vector.tensor_mul(adv[:], mt[:], ssdf[:])
                        tn = state.tile([1, R_TILE], f32, name=f"t{j}", tag=f"t{j}")
                        nc.vector.tensor_add(tn[:], tval[j][:], adv[:])
                        an = state.tile([1, R_TILE], f32, name=f"act{j}", tag=f"act{j}")
                        nc.vector.scalar_tensor_tensor(an[:], in0=tn[:], scalar=float(MAX_RAY),
                                                       in1=mt[:], op0=OP.is_lt, op1=OP.mult)
                        adv4 = work.tile([4, R_TILE], f32, name="adv4", tag="adv4")
                        nc.gpsimd.partition_broadcast(adv4[:], adv[:])
                        dp = work.tile([4, R_TILE], f32, name="dp", tag="dp")
                        nc.vector.tensor_mul(dp[:], adv4[:], dirs[j][:])
                        pn = state.tile([4, R_TILE], f32, name=f"pos{j}", tag=f"pos{j}")
                        nc.vector.tensor_add(pn[:], pos[j][:], dp[:])
                        pos[j], tval[j], act[j] = pn, tn, an
                    else:
                        # final mask & masked points out
                        mk = work.tile([1, R_TILE], f32, name="mk", tag="mk")
                        nc.vector.tensor_scalar(mk[:], absd[:], SEPS, None, op0=OP.is_lt)
                        mk3 = work.tile([3, R_TILE], f32, name="mk3", tag="mk3")
                        nc.gpsimd.partition_broadcast(mk3[:], mk[:])
                        ov = work.tile([3, R_TILE], f32, name="ov", tag="ov")
                        nc.vector.tensor_mul(ov[:], mk3[:], p[0:3, :])
                        nc.sync.dma_start(outT[:, R_TILE * j:R_TILE * (j + 1)], ov[:])

    nc.compile()
    return nc


def _get_program(dt_mode="f32"):
    if dt_mode not in _PROG_CACHE:
        _PROG_CACHE[dt_mode] = _build_program(dt_mode)
    return _PROG_CACHE[dt_mode]


# ---------------------------------------------------------------- entry point
def kernel(lat_geo, lat_exp, lat_app, W1, b1, W2, b2, W3, b3):
    from concourse.bass_utils import run_bass_kernel_spmd

    in_maps = _host_prep(np.asarray(lat_geo), np.asarray(lat_exp), np.asarray(lat_app),
                         np.asarray(W1), np.asarray(b1), np.asarray(W2),
                         np.asarray(b2), np.asarray(W3), np.asarray(b3))
    nc = _get_program(os.environ.get("PCG_DT_MODE", "f32"))
    res = run_bass_kernel_spmd(nc, in_maps, core_ids=list(range(N_CORES)))
    parts = [np.asarray(res.results[i]["outT"]) for i in range(N_CORES)]
    full = np.concatenate(parts, axis=1).T  # [32768, 3]
    return np.ascontiguousarray(full.astype(np.float32))


# revision 8
# speedup vs baseline: 1.1278x; 1.1278x over previous
"""Trainium2 Bass kernel for nn_PointCloudGenerator (neural-SDF sphere tracing).

Strategy
--------
Data-parallel over rays: 2 views x 128x128 rays = 32768 rays, sharded 4096
per NeuronCore across 8 cores.  The MLP is tiny and replicated.

Algorithmic folding done on the host (inside kernel(), plain numpy):
  * The 484-dim latent is constant across rays, so lat @ W1[3:,:] folds into
    the layer-1 bias:  b1' = b1 + lat @ W1[3:].  Layer 1 collapses from a
    487x256 matmul to a 4x256 one (3 pos dims + a ones-row carrying b1').
  * Only W3[:,0] (the SDF head) is ever used - the color head is dead code.
  * Everything sdf-related is pre-scaled by SCALE=sqrt(2) so the advance
    step needs no extra multiply:  ssdf = SCALE*sdf, threshold SCALE*eps.

On-device layout is feature-major: activations live as [features, rays]
tiles so W1/W2/W3 feed the PE array directly as stationary operands with
zero transposes anywhere.
"""

import os
import sys

import numpy as np

# ---------------------------------------------------------------- constants
PU = PV = 128
N_RAYS = PU * PV
FOCAL = 1.0
CAM_DIST = 2.2
MAX_RAY = 3.5
EPS = 1e-3
SCALE = float(np.sqrt(2.0))
STEPS = 6
HID = 256
SPHERE_R = 0.7

N_CORES = 8
R_TOTAL = 2 * N_RAYS          # 32768 rays over both camera angles
R_CORE = R_TOTAL // N_CORES   # 4096
R_TILE = 512
N_TILES = R_CORE // R_TILE    # 8

SEPS = SCALE * EPS

for _p in ("/opt/trn_rl_repo", "/root/.axon_site/_ro/trn_rl_repo"):
    if os.path.isdir(_p) and _p not in sys.path:
        sys.path.append(_p)

_PROG_CACHE = {}


# ---------------------------------------------------------------- host math
def _rays_numpy(rho_deg):
    """Mirror of reference._rays in float32 numpy."""
    u = ((np.arange(PU, dtype=np.float32) + 0.5) / np.float32(PU) - np.float32(0.5)) * np.float32(PU / PV)
    v = np.float32(0.5) - (np.arange(PV, dtype=np.float32) + 0.5) / np.float32(PV)
    uu, vv = np.meshgrid(u, v, indexing="ij")
    d = np.stack([uu, vv, np.full_like(uu, -np.float32(FOCAL))], axis=-1).reshape(-1, 3)
    d = d / np.linalg.norm(d, axis=-1, keepdims=True).astype(np.float32)
    th = np.deg2rad(rho_deg)
    c, s = np.float32(np.cos(th)), np.float32(np.sin(th))
    R = np.array([[c, 0.0, s], [0.0, 1.0, 0.0], [-s, 0.0, c]], dtype=np.float32)
    dirs = (d @ R.T).astype(np.float32)
    cam = (R @ np.array([0.0, 0.0, 1.0], np.float32)) * np.float32(CAM_DIST)
    first_step = np.float32(FOCAL + CAM_DIST - 1.0)
    start = (cam[None, :] + first_step * dirs).astype(np.float32)
    return start, dirs


def _host_prep(lat_geo, lat_exp, lat_app, W1, b1, W2, b2, W3, b3):
    """Fold latents/biases; build per-core device input dicts."""
    f = np.float32
    lat = np.concatenate([lat_geo, lat_exp, lat_app]).astype(f)
    b1p = (b1.astype(f) + lat @ W1[3:, :].astype(f)).astype(f)
    # [4, 256]: rows x,y,z of W1 plus the folded bias row (pos row 3 == 1.0)
    w1a4 = np.concatenate([W1[:3, :].astype(f), b1p[None, :]], axis=0).astype(f)
    w3s = (SCALE * W3[:, 0:1].astype(f)).astype(f)          # [256,1]
    cconst = f(SCALE * SPHERE_R - SCALE * float(b3[0]))     # ssdf = s*|pos| - cconst + h2@w3s
    onesr2 = np.array([[SCALE * SCALE]] * 3 + [[0.0]], f)   # [4,1]: r2 matmul weights

    starts, dirs = [], []
    for rho in (-30.0, 30.0):
        st, di = _rays_numpy(rho)
        starts.append(st)
        dirs.append(di)
    start = np.concatenate(starts, 0)   # [32768, 3]
    dirv = np.concatenate(dirs, 0)

    # feature-major with ones / zeros 4th row
    startT = np.concatenate([start.T, np.ones((1, R_TOTAL), f)], 0).astype(f)  # [4, 32768]
    dirT = np.concatenate([dirv.T, np.zeros((1, R_TOTAL), f)], 0).astype(f)

    in_maps = []
    for c in range(N_CORES):
        sl = slice(c * R_CORE, (c + 1) * R_CORE)
        in_maps.append({
            "startT": np.ascontiguousarray(startT[:, sl]),
            "dirT": np.ascontiguousarray(dirT[:, sl]),
            "w1a4": w1a4,
            "w2": np.ascontiguousarray(W2.astype(f)),
            "w3s": w3s,
            "b2d": np.ascontiguousarray(b2.astype(f).reshape(HID, 1)),
            "onesr2": onesr2,
            "cconst": np.array([[cconst]], f),
        })
    return in_maps


# ---------------------------------------------------------------- device build
def _build_program(dt_mode="f32"):
    """Build the per-core Bass program (SPMD: identical on all 8 cores)."""
    from contextlib import ExitStack

    import concourse.bass as bass  # noqa: F401
    import concourse.tile as tile
    from concourse import bacc, mybir
    from concourse import library_config

    f32 = mybir.dt.float32
    AF = mybir.ActivationFunctionType
    OP = mybir.AluOpType

    f32r = mybir.dt.float32r
    use_r_small = dt_mode == "f32r"           # L1 + r2 matmuls
    use_r_big = dt_mode in ("f32r", "mix")    # L2 + L3 matmuls
    dt_small = f32r if use_r_small else f32   # pos/sq operand tiles
    dt_big = f32r if use_r_big else f32       # h1/h2 operand tiles

    nc = bacc.Bacc("TRN2", target_bir_lowering=False, debug=False,
                   num_devices=N_CORES)

    startT = nc.dram_tensor("startT", [4, R_CORE], f32, kind="ExternalInput").ap()
    dirTd = nc.dram_tensor("dirT", [4, R_CORE], f32, kind="ExternalInput").ap()
    w1a4 = nc.dram_tensor("w1a4", [4, HID], f32, kind="ExternalInput").ap()
    w2 = nc.dram_tensor("w2", [HID, HID], f32, kind="ExternalInput").ap()
    w3s = nc.dram_tensor("w3s", [HID, 1], f32, kind="ExternalInput").ap()
    b2d = nc.dram_tensor("b2d", [HID, 1], f32, kind="ExternalInput").ap()
    onesr2 = nc.dram_tensor("onesr2", [4, 1], f32, kind="ExternalInput").ap()
    cconst = nc.dram_tensor("cconst", [1, 1], f32, kind="ExternalInput").ap()
    outT = nc.dram_tensor("outT", [3, R_CORE], f32, kind="ExternalOutput").ap()

    with tile.TileContext(nc) as tc:
        with ExitStack() as ctx:
            nc.gpsimd.load_library(library_config.attn)

            cpool = ctx.enter_context(tc.tile_pool(name="consts", bufs=1))
            w1a4_s = cpool.tile([4, HID], f32, name="w1a4", tag="w1a4")
            nc.sync.dma_start(w1a4_s[:], w1a4[:])
            w2_s = [[cpool.tile([128, 128], f32, name=f"w2_{i}{j}", tag=f"w2_{i}{j}") for j in range(2)]
                    for i in range(2)]
            for i in range(2):
                for j in range(2):
                    nc.sync.dma_start(w2_s[i][j][:],
                                      w2[128 * i:128 * (i + 1), 128 * j:128 * (j + 1)])
            w3s_s = [cpool.tile([128, 1], f32, name=f"w3s{i}", tag=f"w3s{i}") for i in range(2)]
            b2_s = [cpool.tile([128, 1], f32, name=f"b2{i}", tag=f"b2{i}") for i in range(2)]
            for i in range(2):
                nc.sync.dma_start(w3s_s[i][:], w3s[128 * i:128 * (i + 1), :])
                nc.sync.dma_start(b2_s[i][:], b2d[128 * i:128 * (i + 1), :])
            onesr2_s = cpool.tile([4, 1], f32, name="onesr2", tag="onesr2")
            nc.sync.dma_start(onesr2_s[:], onesr2[:])
            cconst_s = cpool.tile([1, 1], f32, name="cconst", tag="cconst")
            nc.sync.dma_start(cconst_s[:], cconst[:])

            if use_r_small:
                # fp32r matmul operands must be explicitly rounded by their
                # producer op; make rounded copies of the stationary weights.
                w1a4_r = cpool.tile([4, HID], f32r, name="w1a4r", tag="w1a4r")
                nc.vector.tensor_copy(w1a4_r[:], w1a4_s[:])
                onesr2_r = cpool.tile([4, 1], f32r, name="onesr2r", tag="onesr2r")
                nc.vector.tensor_copy(onesr2_r[:], onesr2_s[:])
                w1a4_s, onesr2_s = w1a4_r, onesr2_r
            if use_r_big:
                w2_rb = [[cpool.tile([128, 128], f32r, name=f"w2b_{i}{j}", tag=f"w2b_{i}{j}")
                          for j in range(2)] for i in range(2)]
                for i in range(2):
                    for j in range(2):
                        nc.vector.tensor_copy(w2_rb[i][j][:], w2_s[i][j][:])
                w3s_rb = [cpool.tile([128, 1], f32r, name=f"w3sb{i}", tag=f"w3sb{i}")
                          for i in range(2)]
                for i in range(2):
                    nc.vector.tensor_copy(w3s_rb[i][:], w3s_s[i][:])
                w2_s, w3s_s = w2_rb, w3s_rb

            state = ctx.enter_context(tc.tile_pool(name="state", bufs=2))
            dpool = ctx.enter_context(tc.tile_pool(name="dirs", bufs=1))
            pos, tval, act, dirs = {}, {}, {}, {}
            for j in range(N_TILES):
                sl = slice(R_TILE * j, R_TILE * (j + 1))
                p = state.tile([4, R_TILE], f32, name=f"pos{j}", tag=f"pos{j}")
                nc.sync.dma_start(p[:], startT[:, sl])
                d = dpool.tile([4, R_TILE], f32, name=f"dir{j}", tag=f"dir{j}")
                nc.sync.dma_start(d[:], dirTd[:, sl])
                t0 = state.tile([1, R_TILE], f32, name=f"t{j}", tag=f"t{j}")
                nc.vector.memset(t0[:], 0.0)
                a0 = state.tile([1, R_TILE], f32, name=f"act{j}", tag=f"act{j}")
                nc.vector.memset(a0[:], 1.0)
                pos[j], tval[j], act[j], dirs[j] = p, t0, a0, d

            work = ctx.enter_context(tc.tile_pool(name="work", bufs=2))
            psA = ctx.enter_context(tc.tile_pool(name="psA", bufs=2, space="PSUM"))
            psB = ctx.enter_context(tc.tile_pool(name="psB", bufs=2, space="PSUM"))
            psS = ctx.enter_context(tc.tile_pool(name="psS", bufs=2, space="PSUM"))

            for s in range(STEPS + 1):
                last = s == STEPS
                for j in range(N_TILES):
                    p = pos[j]
                    if use_r_small:
                        pr = work.tile([4, R_TILE], f32r, name="pr", tag="pr")
                        nc.scalar.activation(pr[:], p[:], AF.Copy)
                    else:
                        pr = p
                    # ---- layer 1: h1 = relu(W1a'.pos + b1')   [256, R]
                    h1 = [work.tile([128, R_TILE], dt_big, name=f"h1{m}", tag=f"h1{m}") for m in range(2)]
                    for m in range(2):
                        ph1 = psA.tile([128, R_TILE], f32, name="ph1", tag="ph1")
                        nc.tensor.matmul(ph1[:], lhsT=(w1a4_s[:, 128 * m:128 * (m + 1)]),
                                         rhs=(pr[:]), start=True, stop=True)
                        nc.scalar.activation(h1[m][:], ph1[:], AF.Relu)
                    # ---- scaled norm: s*|pos| via sq -> ones-matmul -> sqrt
                    sq = work.tile([4, R_TILE], dt_small, name="sq", tag="sq")
                    nc.scalar.activation(sq[:], p[:], AF.Square)
                    pr2 = psS.tile([1, R_TILE], f32, name="pr2", tag="pr2")
                    nc.tensor.matmul(pr2[:], lhsT=(onesr2_s[:]), rhs=(sq[:]),
                                     start=True, stop=True)
                    snorm = work.tile([1, R_TILE], f32, name="snorm", tag="snorm")
                    nc.scalar.activation(snorm[:], pr2[:], AF.Sqrt)
                    # ---- layer 2: h2 = relu(W2.h1 + b2)   [256, R]
                    h2 = [work.tile([128, R_TILE], dt_big, name=f"h2{m}", tag=f"h2{m}") for m in range(2)]
                    for m in range(2):
                        ph2 = psB.tile([128, R_TILE], f32, name="ph2", tag="ph2")
                        nc.tensor.matmul(ph2[:], lhsT=(w2_s[0][m][:]), rhs=(h1[0][:]),
                                         start=True, stop=False)
                        nc.tensor.matmul(ph2[:], lhsT=(w2_s[1][m][:]), rhs=(h1[1][:]),
                                         start=False, stop=True)
                        nc.scalar.activation(h2[m][:], ph2[:], AF.Relu, bias=b2_s[m][:])
                    # ---- layer 3 (sdf head only): po = s*(h2 @ W3[:,0])
                    po = psS.tile([1, R_TILE], f32, name="po", tag="po")
                    nc.tensor.matmul(po[:], lhsT=(w3s_s[0][:]), rhs=(h2[0][:]),
                                     start=True, stop=False)
                    nc.tensor.matmul(po[:], lhsT=(w3s_s[1][:]), rhs=(h2[1][:]),
                                     start=False, stop=True)
                    # ---- ssdf = (s*|pos| - cconst) + po
                    ssdf = work.tile([1, R_TILE], f32, name="ssdf", tag="ssdf")
                    nc.vector.scalar_tensor_tensor(ssdf[:], in0=snorm[:], scalar=cconst_s[:],
                                                   in1=po[:], op0=OP.subtract, op1=OP.add)
                    absd = work.tile([1, R_TILE], f32, name="absd", tag="absd")
                    nc.scalar.activation(absd[:], ssdf[:], AF.Abs)
                    if not last:
                        # m = active & ~hit ; adv = m * ssdf
                        mt = work.tile([1, R_TILE], f32, name="mt", tag="mt")
                        nc.vector.scalar_tensor_tensor(mt[:], in0=absd[:], scalar=SEPS,
                                                       in1=act[j][:], op0=OP.is_ge, op1=OP.mult)
                        adv = work.tile([1, R_TILE], f32, name="adv", tag="adv")
                        nc.vector.tensor_mul(adv[:], mt[:], ssdf[:])
                        tn = state.tile([1, R_TILE], f32, name=f"t{j}", tag=f"t{j}")
                        nc.vector.tensor_add(tn[:], tval[j][:], adv[:])
                        an = state.tile([1, R_TILE], f32, name=f"act{j}", tag=f"act{j}")
                        nc.vector.scalar_tensor_tensor(an[:], in0=tn[:], scalar=float(MAX_RAY),
                                                       in1=mt[:], op0=OP.is_lt, op1=OP.mult)
                        adv4 = work.tile([4, R_TILE], f32, name="adv4", tag="adv4")
                        nc.gpsimd.partition_broadcast(adv4[:], adv[:])
                        dp = work.tile([4, R_TILE], f32, name="dp", tag="dp")
                        nc.vector.tensor_mul(dp[:], adv4[:], dirs[j][:])
                        pn = state.tile([4, R_TILE], f32, name=f"pos{j}", tag=f"pos{j}")
                        nc.vector.tensor_add(pn[:], pos[j][:], dp[:])
                        pos[j], tval[j], act[j] = pn, tn, an
                    else:
                        # final mask & masked points out
                        mk = work.tile([1, R_TILE], f32, name="mk", tag="mk")
                        nc.vector.tensor_scalar(mk[:], absd[:], SEPS, None, op0=OP.is_lt)
                        mk3 = work.tile([3, R_TILE], f32, name="mk3", tag="mk3")
                        nc.gpsimd.partition_broadcast(mk3[:], mk[:])
                        ov = work.tile([3, R_TILE], f32, name="ov", tag="ov")
                        nc.vector.tensor_mul(ov[:], mk3[:], p[0:3, :])
                        nc.sync.dma_start(outT[:, R_TILE * j:R_TILE * (j + 1)], ov[:])

    nc.compile()
    return nc


def _get_program(dt_mode="f32"):
    if dt_mode not in _PROG_CACHE:
        _PROG_CACHE[dt_mode] = _build_program(dt_mode)
    return _PROG_CACHE[dt_mode]


# ---------------------------------------------------------------- entry point
def kernel(lat_geo, lat_exp, lat_app, W1, b1, W2, b2, W3, b3):
    from concourse.bass_utils import run_bass_kernel_spmd

    in_maps = _host_prep(np.asarray(lat_geo), np.asarray(lat_exp), np.asarray(lat_app),
                         np.asarray(W1), np.asarray(b1), np.asarray(W2),
                         np.asarray(b2), np.asarray(W3), np.asarray(b3))
    nc = _get_program(os.environ.get("PCG_DT_MODE", "f32"))
    res = run_bass_kernel_spmd(nc, in_maps, core_ids=list(range(N_CORES)))
    parts = [np.asarray(res.results[i]["outT"]) for i in range(N_CORES)]
    full = np.concatenate(parts, axis=1).T  # [32768, 3]
    return np.ascontiguousarray(full.astype(np.float32))
